# revision 1
# baseline (speedup 1.0000x reference)
"""Trainium2 Bass kernel for the LocalAggregator nn.Module.

Reference computation:
    power[p,g]  = -0.5 * d^T Prec_g d          (d = pts[p] - means3D[g])
    within[p,g] = all(|voxel(pts[p]) - voxel(means3D[g])| <= radii[g])
    logits      = where(within & power<=0, exp(power), 0) @ opacities

Device algorithm (everything O(P*G) runs on the NeuronCores):
  * power is a quadratic polynomial in the point coordinates, so it is a
    matmul of 10 point features [x2,y2,z2,xy,yz,xz,x,y,z,1] against
    per-gaussian coefficient columns.
  * the voxel box test is computed EXACTLY by a matmul of one-hot voxel
    index features (value 224) against per-gaussian box indicator columns
    {0,1}: the contribution is 224 * (#axes within).  Folding -3*224 into
    the constant coefficient makes the PSUM accumulator equal
        power + 224*(#within - 3)
    which is exactly `power` for fully-within pairs and <= -224 otherwise,
    so exp() underflows to exactly 0.0 in fp32 (matches the reference's
    hard mask; valid because Prec is PSD so power <= 0).
  * ScalarE evaluates exp from PSUM, then a second matmul contracts the
    weights against opacities:  logits^T[c,p] += opa^T . weight^T.

Sharding: points are sorted into 4 x-slabs x 2 y-halves (2048 points per
core); each core keeps only the gaussians whose voxel box overlaps its
point bounding box (~300-500 of 2048).  Coordinates are re-centered per
core to keep the fp32 quadratic-form cancellation error small.
One-hot rows are packed into the spare rows of the fp32 feature chunk
first; the remainder spills into fp8 chunks of 128 rows.
"""

import numpy as np
import ml_dtypes

import concourse.bass as bass
import concourse.mybir as mybir
import concourse.tile as tile
import concourse.bass2jax as _bass2jax
import concourse.bass_utils as _bass_utils
from concourse.bass_utils import run_bass_kernel_spmd

import json as _json


def _split_waits(bir_json):
    """Walrus in this toolchain rejects instructions carrying more than one
    sync wait ("Too many sync wait commands").  Split every multi-wait
    instruction into a chain of single-wait NoOps on the same engine (program
    order on the engine's sequencer preserves the wait-before-op semantics)."""
    if isinstance(bir_json, (bytes, bytearray)):
        m = _json.loads(bir_json.decode())
    else:
        m = _json.loads(bir_json)
    cnt = 0
    for f in m["functions"]:
        for bb in f["blocks"]:
            new_insts = []
            for inst in bb["instructions"]:
                si = inst.get("sync_info")
                waits = (si or {}).get("on_wait") or []
                if len(waits) > 1:
                    eng = inst.get("engine")
                    for w in waits[:-1]:
                        cnt += 1
                        nop = {
                            "debug": 16,
                            "ins": [],
                            "name": f"I-nopw-{cnt}",
                            "opcode": "NoOp",
                            "outs": [],
                            "sync_info": {"on_update": [], "on_wait": [w]},
                        }
                        if eng is not None:
                            nop["engine"] = eng
                        new_insts.append(nop)
                    si["on_wait"] = [waits[-1]]
                new_insts.append(inst)
            bb["instructions"] = new_insts
    return _json.dumps(m).encode()


_orig_compile_bir_kernel = _bass_utils.compile_bir_kernel.__wrapped__ if hasattr(
    _bass_utils.compile_bir_kernel, "__wrapped__") else _bass_utils.compile_bir_kernel


def _patched_compile_bir_kernel(bir_json, tmpdir, neff_name="file.neff"):
    return _orig_compile_bir_kernel(_split_waits(bir_json), tmpdir, neff_name)


_bass2jax.compile_bir_kernel = _patched_compile_bir_kernel
_bass_utils.compile_bir_kernel = _patched_compile_bir_kernel

GRID = np.float32(0.5)
SCALE_MULT = np.float32(3.0)
MPEN = 224.0  # penalty unit; exact in float8_e4m3 (max 240) and >> 104 (fp32 exp underflow)
N_CORES = 8
FP8_NP = ml_dtypes.float8_e4m3
NQUAD = 10  # quadratic feature rows in chunk 0
PBLK = 1024  # point block per exp/psum tile (2 PSUM banks)
NMM = 512  # matmul moving free dim (fp32 max)

_nc_cache = {}


def _build_bass(P_loc, G_loc, C, n_fp8):
    f32 = mybir.dt.float32
    fp8 = mybir.dt.float8e4
    GT = G_loc // 128
    PCC = P_loc // PBLK
    HB = PBLK // NMM  # halves per point block

    nc = bass.Bass()
    f0_d = nc.dram_tensor("f0", [128, P_loc], f32, kind="ExternalInput")
    w0_d = nc.dram_tensor("w0", [128, G_loc], f32, kind="ExternalInput")
    if n_fp8:
        f1_d = nc.dram_tensor("f1", [128, n_fp8, P_loc], fp8, kind="ExternalInput")
        w1_d = nc.dram_tensor("w1", [128, n_fp8, G_loc], fp8, kind="ExternalInput")
    opa_d = nc.dram_tensor("opa", [128, GT, C], mybir.dt.bfloat16, kind="ExternalInput")
    out_d = nc.dram_tensor("out", [C, P_loc], f32, kind="ExternalOutput")

    with tile.TileContext(nc) as tc:
        with (
            tc.tile_pool(name="singles", bufs=1) as singles,
            tc.tile_pool(name="wpool", bufs=3) as wpool,
            tc.tile_pool(name="opool", bufs=2) as opool,
            tc.tile_pool(name="pp", bufs=2, space="PSUM") as pp,
            tc.tile_pool(name="pl", bufs=2, space="PSUM") as pl,
        ):
            w0_sb = singles.tile([128, G_loc], f32)
            nc.sync.dma_start(out=w0_sb[:], in_=w0_d[:])
            if n_fp8:
                w1_sb = singles.tile([128, n_fp8, G_loc], fp8)
                nc.sync.dma_start(out=w1_sb[:], in_=w1_d[:])
                f1_sb = singles.tile([128, n_fp8, P_loc], fp8)
                nc.sync.dma_start(out=f1_sb[:], in_=f1_d[:])
            opa_sb = singles.tile([128, GT, C], mybir.dt.bfloat16)
            nc.sync.dma_start(out=opa_sb[:], in_=opa_d[:])
            f0_sb = singles.tile([128, P_loc], f32)
            for pcc in range(PCC):
                sl = slice(pcc * PBLK, (pcc + 1) * PBLK)
                nc.sync.dma_start(out=f0_sb[:, sl], in_=f0_d[:, sl])

            for pcc in range(PCC):
                psl = [pl.tile([C, NMM], f32, name=f"psl{h}") for h in range(HB)]
                for gt in range(GT):
                    gsl = slice(gt * 128, (gt + 1) * 128)
                    psp = pp.tile([128, PBLK], f32, name="psp")
                    nch = 1 + n_fp8
                    for h in range(HB):
                        fsl = slice(pcc * PBLK + h * NMM, pcc * PBLK + (h + 1) * NMM)
                        osl = slice(h * NMM, (h + 1) * NMM)
                        nc.tensor.matmul(
                            psp[:, osl], w0_sb[:, gsl], f0_sb[:, fsl],
                            start=True, stop=(nch == 1),
                        )
                    for j in range(n_fp8):
                        for h in range(HB):
                            fsl = slice(pcc * PBLK + h * NMM, pcc * PBLK + (h + 1) * NMM)
                            osl = slice(h * NMM, (h + 1) * NMM)
                            nc.tensor.matmul(
                                psp[:, osl], w1_sb[:, j, gsl], f1_sb[:, j, fsl],
                                start=False, stop=(j == n_fp8 - 1),
                            )
                    wt = wpool.tile([128, PBLK], mybir.dt.bfloat16, name="wt")
                    nc.scalar.activation(
                        out=wt[:], in_=psp[:], func=mybir.ActivationFunctionType.Exp
                    )
                    for h in range(HB):
                        osl = slice(h * NMM, (h + 1) * NMM)
                        nc.tensor.matmul(
                            psl[h][:], opa_sb[:, gt, :], wt[:, osl],
                            start=(gt == 0), stop=(gt == GT - 1),
                        )
                for h in range(HB):
                    osb = opool.tile([C, NMM], f32, name="osb")
                    nc.vector.tensor_copy(out=osb[:], in_=psl[h][:])
                    osl = slice(pcc * PBLK + h * NMM, pcc * PBLK + (h + 1) * NMM)
                    nc.sync.dma_start(out=out_d[:, osl], in_=osb[:])
    return nc


def _prepare(inputs):
    """Host-side O(P+G) prep: sharding, feature/coefficient matrices."""
    pts = np.ascontiguousarray(np.asarray(inputs["pts"], dtype=np.float32))
    means3D = np.ascontiguousarray(np.asarray(inputs["means3D"], dtype=np.float32))
    opac = np.asarray(inputs["opacities"], dtype=np.float32)
    scales = np.asarray(inputs["scales"], dtype=np.float32)
    cov3D = np.asarray(inputs["cov3D"], dtype=np.float32)
    pc_min = np.asarray(inputs["pc_min"], dtype=np.float32)

    P = pts.shape[0]
    G = means3D.shape[0]
    C = opac.shape[1]
    assert P % N_CORES == 0
    P_loc = P // N_CORES

    # integer voxel quantities, identical fp32 arithmetic to the reference
    pts_int = np.floor((pts - pc_min[None, :]) / GRID).astype(np.int32)
    means_int = np.floor((means3D - pc_min[None, :]) / GRID).astype(np.int32)
    radii = np.ceil(scales.max(-1) * SCALE_MULT / GRID).astype(np.int32)
    cov6 = cov3D.reshape(G, 9)[:, [0, 4, 8, 1, 5, 2]].astype(np.float64)

    # spatial sharding: 4 x-slabs (by sorted order) x 2 y-halves
    order = np.argsort(pts_int[:, 0], kind="stable")
    parts = []
    q = P // 4
    for xs in range(4):
        chunk = order[xs * q:(xs + 1) * q]
        sub = chunk[np.argsort(pts_int[chunk, 1], kind="stable")]
        parts.append(sub[: q // 2])
        parts.append(sub[q // 2:])
    perm = np.concatenate(parts)

    cores = []
    gmax = 1
    spill_max = 0
    for ci in range(N_CORES):
        idx = perm[ci * P_loc:(ci + 1) * P_loc]
        pi = pts_int[idx]
        lo = pi.min(axis=0)
        hi = pi.max(axis=0)
        span = hi - lo + 1  # [Sz... order: axis 0=x,1=y,2=z]
        gsel = np.where(
            (means_int[:, 0] >= lo[0] - radii) & (means_int[:, 0] <= hi[0] + radii)
            & (means_int[:, 1] >= lo[1] - radii) & (means_int[:, 1] <= hi[1] + radii)
            & (means_int[:, 2] >= lo[2] - radii) & (means_int[:, 2] <= hi[2] + radii)
        )[0]
        cores.append((idx, lo, hi, gsel))
        gmax = max(gmax, len(gsel))
        S = int(span.sum())
        spill_max = max(spill_max, S - (128 - NQUAD))
    G_loc = int(np.ceil(gmax / 128) * 128)
    n_fp8 = int(np.ceil(max(0, spill_max) / 128))

    free0 = 128 - NQUAD  # one-hot rows available in the fp32 chunk
    KTOT = 128 + n_fp8 * 128

    def row_of(s):  # flat one-hot index -> feature row
        return np.where(s < free0, NQUAD + s, 128 + (s - free0))

    in_maps = []
    for ci in range(N_CORES):
        idx, lo, hi, gsel = cores[ci]
        npts = len(idx)
        gl = len(gsel)
        span = hi - lo + 1
        # axis order for the flat one-hot space: z, x, y (z smallest)
        axes = [2, 0, 1]
        offs = np.zeros(3, np.int64)
        acc = 0
        for a in axes:
            offs[a] = acc
            acc += int(span[a])

        cen = (lo + hi + 1).astype(np.float64) * (0.5 * float(GRID))  # meters
        p64 = pts[idx].astype(np.float64) - cen
        m64 = means3D[gsel].astype(np.float64) - cen

        FH = np.zeros((KTOT, npts), np.float32)
        x, y, z = p64[:, 0], p64[:, 1], p64[:, 2]
        FH[0] = x * x; FH[1] = y * y; FH[2] = z * z
        FH[3] = x * y; FH[4] = y * z; FH[5] = x * z
        FH[6] = x; FH[7] = y; FH[8] = z; FH[9] = 1.0
        tcol = np.arange(npts)
        for a in axes:
            s = offs[a] + (pts_int[idx, a] - lo[a])
            FH[row_of(s), tcol] = MPEN

        WH = np.zeros((KTOT, G_loc), np.float32)
        a_, b_, c_ = cov6[gsel, 0], cov6[gsel, 1], cov6[gsel, 2]
        pxy, pyz, pxz = cov6[gsel, 3], cov6[gsel, 4], cov6[gsel, 5]
        mx, my, mz = m64[:, 0], m64[:, 1], m64[:, 2]
        Amx = a_ * mx + pxy * my + pxz * mz
        Amy = pxy * mx + b_ * my + pyz * mz
        Amz = pxz * mx + pyz * my + c_ * mz
        mAm = mx * Amx + my * Amy + mz * Amz
        WH[0, :gl] = -0.5 * a_; WH[1, :gl] = -0.5 * b_; WH[2, :gl] = -0.5 * c_
        WH[3, :gl] = -pxy; WH[4, :gl] = -pyz; WH[5, :gl] = -pxz
        WH[6, :gl] = Amx; WH[7, :gl] = Amy; WH[8, :gl] = Amz
        WH[9, :gl] = -0.5 * mAm - 3.0 * MPEN
        WH[9, gl:] = -3.0 * MPEN  # padded gaussians: exp(-672) == 0
        for a in axes:
            Sa = int(span[a])
            blo = means_int[gsel, a] - radii[gsel] - lo[a]
            bhi = means_int[gsel, a] + radii[gsel] - lo[a]
            k = np.arange(Sa)[:, None]
            box = ((k >= blo[None, :]) & (k <= bhi[None, :])).astype(np.float32)
            WH[row_of(offs[a] + np.arange(Sa))[:, None], np.arange(gl)[None, :]] = box

        opa_pad = np.zeros((G_loc, C), np.float32)
        opa_pad[:gl] = opac[gsel]

        m = {
            "f0": np.ascontiguousarray(FH[:128]),
            "w0": np.ascontiguousarray(WH[:128]),
            "opa": np.ascontiguousarray(
                opa_pad.reshape(G_loc // 128, 128, C).transpose(1, 0, 2)
            ).astype(ml_dtypes.bfloat16),
        }
        if n_fp8:
            m["f1"] = np.ascontiguousarray(
                FH[128:].reshape(n_fp8, 128, npts).transpose(1, 0, 2)
            ).astype(FP8_NP)
            m["w1"] = np.ascontiguousarray(
                WH[128:].reshape(n_fp8, 128, G_loc).transpose(1, 0, 2)
            ).astype(FP8_NP)
        in_maps.append(m)

    return in_maps, perm, (P, P_loc, G_loc, C, n_fp8)


def _run(inputs, trace=False, **run_kwargs):
    in_maps, perm, (P, P_loc, G_loc, C, n_fp8) = _prepare(inputs)
    key = (P_loc, G_loc, C, n_fp8)
    if key not in _nc_cache:
        _nc_cache[key] = _build_bass(P_loc, G_loc, C, n_fp8)
    nc = _nc_cache[key]
    try:
        res = run_bass_kernel_spmd(
            nc, in_maps, core_ids=list(range(N_CORES)), trace=trace, **run_kwargs
        )
    except ModuleNotFoundError:
        res = run_bass_kernel_spmd(
            nc, in_maps, core_ids=list(range(N_CORES)), trace=False, **run_kwargs
        )
    out = np.empty((P, C), np.float32)
    for ci in range(N_CORES):
        out[perm[ci * P_loc:(ci + 1) * P_loc]] = res.results[ci]["out"].T
    return out, res


def kernel(**inputs):
    return _run(inputs)[0]



# revision 3
# speedup vs baseline: 3.7036x; 3.7036x over previous
"""Trainium2 Bass kernel for the LocalAggregator nn.Module.

Reference computation:
    power[p,g]  = -0.5 * d^T Prec_g d          (d = pts[p] - means3D[g])
    within[p,g] = all(|voxel(pts[p]) - voxel(means3D[g])| <= radii[g])
    logits      = where(within & power<=0, exp(power), 0) @ opacities

Device algorithm:
  * Points are split into 128-point spatial blocks by a recursive KD
    median split; each core owns 16 blocks.  Per block only the
    gaussians whose voxel box overlaps the block bbox are kept
    (<=G_pad of 2048), so the dense pair work per core is
    16 * 128 * G_pad instead of 2048 * 2048.
  * power is a quadratic polynomial in the point coordinates:
    matmul of per-point quadratic features against per-(block,gaussian)
    coefficient columns.  Both sides are stored as two-level fp16
    splits (hi+mid); the three >=2^-22 cross products [Qh*Wh + Qh*Wm +
    Qm*Wh] are computed by stacking rows, so a single 1-cycle/row fp16
    matmul gives ~22-bit precision.
  * the voxel box test is folded into the same matmul with one-hot
    rows over (voxel - lo) mod 16 per axis: contribution 224 per
    within-axis, with -3*224 folded into the constant coefficient, so
    not-within pairs get power <= -224+eps and exp underflows to 0 in
    fp32 (matching the reference's hard mask; Prec is PSD so true
    power <= 0).  The mod-16 aliasing is safe: an aliased pair is
    >=5 m away on that axis, so power <= -25 and exp(power) < 1e-11.
  * ScalarE evaluates exp from PSUM into fp16 weights; a second matmul
    per 128-point block contracts weights against opacities with the
    points as the PSUM partition axis:  logits[p, c] += wt^T . opa.
    All 16 blocks' logits live in a single PSUM bank; one DVE copy and
    one DMA drain the core's whole output.
"""

import numpy as np

import concourse.bass as bass
import concourse.mybir as mybir
import concourse.tile as tile
import concourse.bass2jax as _bass2jax
import concourse.bass_utils as _bass_utils
from concourse.bass_utils import run_bass_kernel_spmd

import json as _json


def _split_waits(bir_json):
    """Walrus in this toolchain rejects instructions carrying more than one
    sync wait ("Too many sync wait commands").  Split every multi-wait
    instruction into a chain of single-wait NoOps on the same engine (program
    order on the engine's sequencer preserves the wait-before-op semantics)."""
    if isinstance(bir_json, (bytes, bytearray)):
        m = _json.loads(bir_json.decode())
    else:
        m = _json.loads(bir_json)
    cnt = 0
    for f in m["functions"]:
        for bb in f["blocks"]:
            new_insts = []
            for inst in bb["instructions"]:
                si = inst.get("sync_info")
                waits = (si or {}).get("on_wait") or []
                if len(waits) > 1:
                    eng = inst.get("engine")
                    for w in waits[:-1]:
                        cnt += 1
                        nop = {
                            "debug": 16,
                            "ins": [],
                            "name": f"I-nopw-{cnt}",
                            "opcode": "NoOp",
                            "outs": [],
                            "sync_info": {"on_update": [], "on_wait": [w]},
                        }
                        if eng is not None:
                            nop["engine"] = eng
                        new_insts.append(nop)
                    si["on_wait"] = [waits[-1]]
                new_insts.append(inst)
            bb["instructions"] = new_insts
    return _json.dumps(m).encode()


_orig_compile_bir_kernel = _bass_utils.compile_bir_kernel.__wrapped__ if hasattr(
    _bass_utils.compile_bir_kernel, "__wrapped__") else _bass_utils.compile_bir_kernel


def _patched_compile_bir_kernel(bir_json, tmpdir, neff_name="file.neff"):
    return _orig_compile_bir_kernel(_split_waits(bir_json), tmpdir, neff_name)


_bass2jax.compile_bir_kernel = _patched_compile_bir_kernel
_bass_utils.compile_bir_kernel = _patched_compile_bir_kernel

GRID = np.float64(0.5)
SCALE_MULT = np.float64(3.0)
MPEN = 224.0  # penalty unit; exact in fp16, and 224 > 104 (fp32 exp underflow)
N_CORES = 8
PBLK = 128  # points per spatial block
NBC = 16  # blocks per core
MOD = 16  # one-hot modulo per axis (aliased pairs are >=5m away -> exp==0)

_nc_cache = {}


def _build_bass(R, G_pad, C):
    """One core's program.  R = feature rows (<=128), G_pad = gaussian slots
    per block (multiple of 128), C = channels."""
    f16 = mybir.dt.float16
    f32 = mybir.dt.float32
    GT = G_pad // 128
    NHALF = NBC // 2  # blocks per input half
    FCOLS = NHALF * PBLK  # f columns per half
    WCOLS = NHALF * G_pad  # w columns per half
    HCOLS = FCOLS + WCOLS
    jobs_per_half = NHALF * GT  # (block, gtile) pairs; 4 jobs per PSUM bank

    nc = bass.Bass()
    fw_d = nc.dram_tensor("fw", [R, 2 * HCOLS], f16, kind="ExternalInput")
    opa_d = nc.dram_tensor("opa", [128, NBC * GT * C], f16, kind="ExternalInput")
    out_d = nc.dram_tensor("out", [128, NBC * C], f32, kind="ExternalOutput")

    with tile.TileContext(nc) as tc:
        with (
            tc.tile_pool(name="singles", bufs=1) as singles,
            tc.tile_pool(name="pp", bufs=4, space="PSUM") as pp,
            tc.tile_pool(name="pl", bufs=1, space="PSUM") as pl,
        ):
            fw_sb = singles.tile([R, 2 * HCOLS], f16)
            opa_sb = singles.tile([128, NBC * GT * C], f16)
            wt = singles.tile([128, NBC * GT * PBLK], f16)
            osb = singles.tile([128, NBC * C], f32)
            psl = pl.tile([128, NBC * C], f32, name="psl")

            nc.sync.dma_start(out=fw_sb[:, :HCOLS], in_=fw_d[:, :HCOLS])
            nc.sync.dma_start(out=opa_sb[:], in_=opa_d[:])
            nc.sync.dma_start(out=fw_sb[:, HCOLS:], in_=fw_d[:, HCOLS:])

            njobs = 2 * jobs_per_half
            job = 0
            for h in range(2):
                base = h * HCOLS
                for q in range((jobs_per_half + 3) // 4):
                    nj = min(4, jobs_per_half - q * 4)
                    psp = pp.tile([128, 4 * PBLK], f32, name="psp")
                    j0 = job
                    for j in range(nj):
                        blk_h, gt = divmod(q * 4 + j, GT)
                        fsl = slice(base + blk_h * PBLK, base + (blk_h + 1) * PBLK)
                        wsl = slice(
                            base + FCOLS + blk_h * G_pad + gt * 128,
                            base + FCOLS + blk_h * G_pad + gt * 128 + 128,
                        )
                        nc.tensor.matmul(
                            psp[:, j * PBLK:(j + 1) * PBLK],
                            fw_sb[:, wsl], fw_sb[:, fsl],
                            start=(j == 0), stop=(j == nj - 1),
                        )
                        job += 1
                    wt_sl = slice(j0 * PBLK, (j0 + nj) * PBLK)
                    nc.scalar.activation(
                        out=wt[:, wt_sl], in_=psp[:, :nj * PBLK],
                        func=mybir.ActivationFunctionType.Exp,
                    )
                    for j in range(nj):
                        gj = j0 + j
                        blk_h, gt = divmod(q * 4 + j, GT)
                        blk = h * NHALF + blk_h
                        nc.tensor.matmul(
                            psl[:, blk * C:(blk + 1) * C],
                            wt[:, gj * PBLK:(gj + 1) * PBLK],
                            opa_sb[:, gj * C:(gj + 1) * C],
                            start=(gj == 0), stop=(gj == njobs - 1),
                        )
            nc.vector.tensor_copy(out=osb[:], in_=psl[:])
            nc.sync.dma_start(out=out_d[:], in_=osb[:])
    return nc


def _kd_blocks(pts_int, n_blocks):
    """Recursive median split on the widest voxel axis -> equal-size blocks."""
    depth = int(np.log2(n_blocks))
    assert (1 << depth) == n_blocks
    blocks = []

    def rec(idx, d):
        if d == 0:
            blocks.append(idx)
            return
        pi = pts_int[idx]
        ax = int(np.argmax(pi.max(0) - pi.min(0)))
        o = idx[np.argsort(pi[:, ax], kind="stable")]
        half = len(o) // 2
        rec(o[:half], d - 1)
        rec(o[half:], d - 1)

    rec(np.arange(len(pts_int)), depth)
    return blocks


def _split16(v):
    """Two-level fp16 split: v ~= hi + mid with ~22-bit mantissa coverage."""
    hi = v.astype(np.float16)
    mid = (v - hi.astype(np.float64)).astype(np.float16)
    return hi, mid


def _prepare(inputs):
    """Host-side prep: KD sharding, per-block gaussian sets, feature and
    coefficient matrices.  All O(P + n_blocks * G)."""
    pts = np.ascontiguousarray(np.asarray(inputs["pts"], dtype=np.float32))
    means3D = np.ascontiguousarray(np.asarray(inputs["means3D"], dtype=np.float32))
    opac = np.asarray(inputs["opacities"], dtype=np.float32)
    scales = np.asarray(inputs["scales"], dtype=np.float32)
    cov3D = np.asarray(inputs["cov3D"], dtype=np.float32)
    pc_min = np.asarray(inputs["pc_min"], dtype=np.float32)

    P = pts.shape[0]
    G = means3D.shape[0]
    C = opac.shape[1]
    n_blocks = N_CORES * NBC
    assert P == n_blocks * PBLK, (P, n_blocks * PBLK)

    # integer voxel quantities, identical fp32 arithmetic to the reference
    pts_int = np.floor((pts - pc_min[None, :]) / np.float32(GRID)).astype(np.int32)
    means_int = np.floor((means3D - pc_min[None, :]) / np.float32(GRID)).astype(np.int32)
    radii = np.ceil(scales.max(-1) * np.float32(SCALE_MULT) / np.float32(GRID)).astype(np.int32)
    cov6 = cov3D.reshape(G, 9)[:, [0, 4, 8, 1, 5, 2]].astype(np.float64)
    has_offdiag = np.abs(cov6[:, 3:]).max() > 0.0
    NQ = 10 if has_offdiag else 7

    blocks = _kd_blocks(pts_int, n_blocks)
    perm = np.concatenate(blocks)

    binfo = []
    gmax = 1
    for blk in blocks:
        pi = pts_int[blk]
        lo = pi.min(0)
        hi = pi.max(0)
        gsel = np.where(
            (means_int[:, 0] >= lo[0] - radii) & (means_int[:, 0] <= hi[0] + radii)
            & (means_int[:, 1] >= lo[1] - radii) & (means_int[:, 1] <= hi[1] + radii)
            & (means_int[:, 2] >= lo[2] - radii) & (means_int[:, 2] <= hi[2] + radii)
        )[0]
        binfo.append((blk, lo, hi, gsel))
        gmax = max(gmax, len(gsel))
    G_pad = int(np.ceil(gmax / 128) * 128)
    GT = G_pad // 128

    R = 3 * NQ + 3 * MOD
    assert R <= 128, R
    NHALF = NBC // 2
    FCOLS = NHALF * PBLK
    WCOLS = NHALF * G_pad
    HCOLS = FCOLS + WCOLS

    bq = np.float64(0.5) * GRID  # voxel center scale

    in_maps = []
    for ci in range(N_CORES):
        fw = np.zeros((R, 2 * HCOLS), np.float16)
        opa_m = np.zeros((128, NBC * GT * C), np.float16)
        for bi in range(NBC):
            blk, lo, hi, gsel = binfo[ci * NBC + bi]
            gl = len(gsel)
            h, blk_h = divmod(bi, NHALF)
            base = h * HCOLS
            fsl = base + blk_h * PBLK
            wsl = base + FCOLS + blk_h * G_pad

            cen = (lo + hi + 1).astype(np.float64) * bq  # block center, meters
            p64 = pts[blk].astype(np.float64) - cen
            m64 = means3D[gsel].astype(np.float64) - cen

            # ---- point features -------------------------------------
            x, y, z = p64[:, 0], p64[:, 1], p64[:, 2]
            if has_offdiag:
                Q = np.stack([x * x, y * y, z * z, x * y, y * z, x * z,
                              x, y, z, np.ones_like(x)])
            else:
                Q = np.stack([x * x, y * y, z * z, x, y, z, np.ones_like(x)])
            Qh, Qm = _split16(Q)
            F = fw[:, fsl:fsl + PBLK]
            F[0:NQ] = Qh
            F[NQ:2 * NQ] = Qh
            F[2 * NQ:3 * NQ] = Qm
            tcol = np.arange(PBLK)
            for a in range(3):
                r = 3 * NQ + a * MOD + ((pts_int[blk, a] - lo[a]) % MOD)
                F[r, tcol] = 1.0

            # ---- gaussian coefficients ------------------------------
            a_, b_, c_ = cov6[gsel, 0], cov6[gsel, 1], cov6[gsel, 2]
            pxy, pyz, pxz = cov6[gsel, 3], cov6[gsel, 4], cov6[gsel, 5]
            mx, my, mz = m64[:, 0], m64[:, 1], m64[:, 2]
            Amx = a_ * mx + pxy * my + pxz * mz
            Amy = pxy * mx + b_ * my + pyz * mz
            Amz = pxz * mx + pyz * my + c_ * mz
            mAm = mx * Amx + my * Amy + mz * Amz
            const = -0.5 * mAm - 3.0 * MPEN
            if has_offdiag:
                Wq = np.stack([-0.5 * a_, -0.5 * b_, -0.5 * c_,
                               -pxy, -pyz, -pxz, Amx, Amy, Amz, const])
            else:
                Wq = np.stack([-0.5 * a_, -0.5 * b_, -0.5 * c_,
                               Amx, Amy, Amz, const])
            Wh, Wm = _split16(Wq)
            W = fw[:, wsl:wsl + G_pad]
            W[0:NQ, :gl] = Wh
            W[NQ:2 * NQ, :gl] = Wm
            W[2 * NQ:3 * NQ, :gl] = Wh
            W[NQ - 1, gl:] = np.float16(-3.0 * MPEN)  # padded: exp(-672)==0
            gc = np.arange(gl)
            for a in range(3):
                blo = means_int[gsel, a] - radii[gsel]
                bhi = means_int[gsel, a] + radii[gsel]
                for v in range(lo[a], hi[a] + 1):
                    r = 3 * NQ + a * MOD + ((v - lo[a]) % MOD)
                    W[r, gc[(blo <= v) & (v <= bhi)]] = np.float16(MPEN)

            # ---- opacities ------------------------------------------
            for gt in range(GT):
                gj = bi * GT + gt
                seg = gsel[gt * 128:(gt + 1) * 128]
                opa_m[: len(seg), gj * C:(gj + 1) * C] = opac[seg].astype(np.float16)

        in_maps.append({"fw": fw, "opa": opa_m})

    return in_maps, perm, (P, G_pad, C, R)


def _run(inputs, trace=False, **run_kwargs):
    in_maps, perm, (P, G_pad, C, R) = _prepare(inputs)
    key = (R, G_pad, C)
    if key not in _nc_cache:
        _nc_cache[key] = _build_bass(R, G_pad, C)
    nc = _nc_cache[key]
    try:
        res = run_bass_kernel_spmd(
            nc, in_maps, core_ids=list(range(N_CORES)), trace=trace, **run_kwargs
        )
    except ModuleNotFoundError:
        res = run_bass_kernel_spmd(
            nc, in_maps, core_ids=list(range(N_CORES)), trace=False, **run_kwargs
        )
    out = np.empty((P, C), np.float32)
    for ci in range(N_CORES):
        o = res.results[ci]["out"]  # [128, NBC*C]
        for bi in range(NBC):
            rows = perm[(ci * NBC + bi) * PBLK:(ci * NBC + bi + 1) * PBLK]
            out[rows] = o[:, bi * C:(bi + 1) * C]
    return out, res


def kernel(**inputs):
    return _run(inputs)[0]


# revision 16
# speedup vs baseline: 3.9289x; 1.0608x over previous
"""Trainium2 Bass kernel for the LocalAggregator nn.Module.

Reference computation:
    power[p,g]  = -0.5 * d^T Prec_g d          (d = pts[p] - means3D[g])
    within[p,g] = all(|voxel(pts[p]) - voxel(means3D[g])| <= radii[g])
    logits      = where(within & power<=0, exp(power), 0) @ opacities

Device algorithm:
  * Points are split into 128-point spatial blocks by a recursive KD
    median split; each core owns 16 blocks.  Per block only the
    gaussians whose voxel box overlaps the block bbox are kept
    (<=G_pad of 2048), so the dense pair work per core is
    16 * 128 * G_pad instead of 2048 * 2048.
  * power is a quadratic polynomial in the point coordinates:
    matmul of per-point quadratic features against per-(block,gaussian)
    coefficient columns.  Both sides are stored as two-level fp16
    splits (hi+mid); the three >=2^-22 cross products [Qh*Wh + Qh*Wm +
    Qm*Wh] are computed by stacking rows, so a single 1-cycle/row fp16
    matmul gives ~22-bit precision.
  * the voxel box test is folded into the same matmul with one-hot
    rows over (voxel - lo) mod 16 per axis: contribution 224 per
    within-axis, with -3*224 folded into the constant coefficient, so
    not-within pairs get power <= -224+eps and exp underflows to 0 in
    fp32 (matching the reference's hard mask; Prec is PSD so true
    power <= 0).  The mod-16 aliasing is safe: an aliased pair is
    >=5 m away on that axis, so power <= -25 and exp(power) < 1e-11.
  * ScalarE evaluates exp from PSUM into fp16 weights; a second matmul
    per 128-point block contracts weights against opacities with the
    points as the PSUM partition axis:  logits[p, c] += wt^T . opa.
    All 16 blocks' logits live in a single PSUM bank; one DVE copy and
    one DMA drain the core's whole output.
"""

import numpy as np

import concourse.bass as bass
import concourse.mybir as mybir
import concourse.tile as tile
import concourse.bass2jax as _bass2jax
import concourse.bass_utils as _bass_utils
from concourse.bass_utils import run_bass_kernel_spmd

import json as _json


def _split_waits(bir_json):
    """Walrus in this toolchain rejects instructions carrying more than one
    sync wait ("Too many sync wait commands").  Split every multi-wait
    instruction into a chain of single-wait NoOps on the same engine (program
    order on the engine's sequencer preserves the wait-before-op semantics)."""
    if isinstance(bir_json, (bytes, bytearray)):
        m = _json.loads(bir_json.decode())
    else:
        m = _json.loads(bir_json)
    cnt = 0
    for f in m["functions"]:
        for bb in f["blocks"]:
            new_insts = []
            for inst in bb["instructions"]:
                si = inst.get("sync_info")
                waits = (si or {}).get("on_wait") or []
                if len(waits) > 1:
                    eng = inst.get("engine")
                    for w in waits[:-1]:
                        cnt += 1
                        nop = {
                            "debug": 16,
                            "ins": [],
                            "name": f"I-nopw-{cnt}",
                            "opcode": "NoOp",
                            "outs": [],
                            "sync_info": {"on_update": [], "on_wait": [w]},
                        }
                        if eng is not None:
                            nop["engine"] = eng
                        new_insts.append(nop)
                    si["on_wait"] = [waits[-1]]
                new_insts.append(inst)
            bb["instructions"] = new_insts
    return _json.dumps(m).encode()


_orig_compile_bir_kernel = _bass_utils.compile_bir_kernel.__wrapped__ if hasattr(
    _bass_utils.compile_bir_kernel, "__wrapped__") else _bass_utils.compile_bir_kernel


def _patched_compile_bir_kernel(bir_json, tmpdir, neff_name="file.neff"):
    return _orig_compile_bir_kernel(_split_waits(bir_json), tmpdir, neff_name)


_bass2jax.compile_bir_kernel = _patched_compile_bir_kernel
_bass_utils.compile_bir_kernel = _patched_compile_bir_kernel

GRID = np.float64(0.5)
SCALE_MULT = np.float64(3.0)
MPEN = 224.0  # penalty unit; exact in fp16, and 224 > 104 (fp32 exp underflow)
N_CORES = 8
PBLK = 128  # points per spatial block
NBC = 16  # blocks per core
MOD = 16  # one-hot modulo per axis (aliased pairs are >=5m away -> exp==0)

_nc_cache = {}
_FORCE128 = False


N_WARM = 0  # dummy matmuls keeping the PE p-state ramp warm during DMA wait
NHB = NBC // 2  # block slots per input half


def _jobs_of(slot_caps):
    """Expand per-slot gaussian capacities into (slot, cap, first, last) jobs
    of <=128 gaussians each.  Jobs are ordered cap-descending so the matmul
    that OPENS each PSUM accumulation region (start=True) covers at least the
    partition range of every later matmul in that region; otherwise rows
    beyond the first matmul's partition count would accumulate onto
    uninitialized PSUM."""
    jobs = []
    for s, cap in enumerate(slot_caps):
        chunks = []
        left = cap
        while left > 0:
            take = min(128, left)
            chunks.append(take)
            left -= take
        for i, ch in enumerate(chunks):
            jobs.append((s, ch, i == 0, i == len(chunks) - 1))
    jobs.sort(key=lambda t: -t[1])
    return jobs


def _build_bass(R, slot_caps, C):
    """One core's program.  R = feature rows (<=128), slot_caps = per-block
    gaussian capacities (len NBC; slots 0..7 = input half 0), C = channels."""
    f16 = mybir.dt.float16
    f32 = mybir.dt.float32
    # per-half fw layout: [f (NHB*PBLK) | w slot (slot_caps)]
    FCOLS = NHB * PBLK
    wstart = {}
    hcols = [0, 0]
    for h in range(2):
        off = FCOLS
        for s in range(h * NHB, (h + 1) * NHB):
            wstart[s] = off
            off += slot_caps[s]
        hcols[h] = off
    TOT = hcols[0] + hcols[1]
    half_jobs = [_jobs_of(slot_caps[:NHB]), _jobs_of(slot_caps[NHB:])]
    njobs = len(half_jobs[0]) + len(half_jobs[1])

    nc = bass.Bass()
    fw_d = nc.dram_tensor("fw", [R, TOT], f16, kind="ExternalInput")
    opa_d = nc.dram_tensor("opa", [128, njobs * C], f16, kind="ExternalInput")
    out_d = nc.dram_tensor("out", [128, NBC * C], f32, kind="ExternalOutput")

    ppbufs = 2 if max(len(half_jobs[0]), len(half_jobs[1])) <= 12 else 1
    with tile.TileContext(nc) as tc:
        with (
            tc.tile_pool(name="singles", bufs=1) as singles,
            tc.tile_pool(name="pp", bufs=ppbufs, space="PSUM") as pp,
            tc.tile_pool(name="pl", bufs=1, space="PSUM") as pl,
        ):
            fw_sb = singles.tile([R, TOT], f16)
            opa_sb = singles.tile([128, njobs * C], f16)
            wt = singles.tile([128, njobs * PBLK], f16)
            osb = singles.tile([128, NBC * C], f32)
            psl = pl.tile([128, NBC * C], f32, name="psl")

            nc.sync.dma_start(out=fw_sb[:, :hcols[0]], in_=fw_d[:, :hcols[0]])
            nc.sync.dma_start(out=fw_sb[:, hcols[0]:], in_=fw_d[:, hcols[0]:])
            nc.sync.dma_start(out=opa_sb[:], in_=opa_d[:])

            gj0 = 0
            for h in range(2):
                base = (0, hcols[0])[h]
                jobs = half_jobs[h]
                nh = len(jobs)
                psp = pp.tile([128, nh * PBLK], f32, name="psp")
                woff = {}
                for j, (s_h, cap, first, last) in enumerate(jobs):
                    s = s_h + h * NHB
                    o = woff.get(s, wstart[s])
                    woff[s] = o + cap
                    blk_h = s_h
                    fsl = slice(base + blk_h * PBLK, base + (blk_h + 1) * PBLK)
                    wsl = slice(base + o, base + o + cap)
                    nc.tensor.matmul(
                        psp[:cap, j * PBLK:(j + 1) * PBLK],
                        fw_sb[:, wsl], fw_sb[:, fsl],
                        start=(j % 4 == 0), stop=(j % 4 == 3 or j == nh - 1),
                    )
                for q0 in range(0, nh, 4):
                    q1 = min(q0 + 4, nh)
                    nc.scalar.activation(
                        out=wt[:, (gj0 + q0) * PBLK:(gj0 + q1) * PBLK],
                        in_=psp[:, q0 * PBLK:q1 * PBLK],
                        func=mybir.ActivationFunctionType.Exp,
                    )
                for j, (s_h, cap, first, last) in enumerate(jobs):
                    s = s_h + h * NHB
                    gj = gj0 + j
                    nc.tensor.matmul(
                        psl[:, s * C:(s + 1) * C],
                        wt[:cap, gj * PBLK:(gj + 1) * PBLK],
                        opa_sb[:cap, gj * C:(gj + 1) * C],
                        start=(j == 0), stop=(j == nh - 1),
                    )
                gj0 += nh
                hsl = slice(h * NHB * C, (h + 1) * NHB * C)
                nc.vector.tensor_copy(out=osb[:, hsl], in_=psl[:, hsl])
                nc.sync.dma_start(out=out_d[:, hsl], in_=osb[:, hsl])
    return nc


def _kd_blocks(pts_int, n_blocks):
    """Recursive median split on the widest voxel axis -> equal-size blocks."""
    depth = int(np.log2(n_blocks))
    assert (1 << depth) == n_blocks
    blocks = []

    def rec(idx, d):
        if d == 0:
            blocks.append(idx)
            return
        pi = pts_int[idx]
        ax = int(np.argmax(pi.max(0) - pi.min(0)))
        o = idx[np.argsort(pi[:, ax], kind="stable")]
        half = len(o) // 2
        rec(o[:half], d - 1)
        rec(o[half:], d - 1)

    rec(np.arange(len(pts_int)), depth)
    return blocks


def _split16(v):
    """Two-level fp16 split: v ~= hi + mid with ~22-bit mantissa coverage."""
    hi = v.astype(np.float16)
    mid = (v - hi.astype(np.float64)).astype(np.float16)
    return hi, mid


def _prepare(inputs):
    """Host-side prep: KD sharding, per-block gaussian sets, feature and
    coefficient matrices.  All O(P + n_blocks * G)."""
    pts = np.ascontiguousarray(np.asarray(inputs["pts"], dtype=np.float32))
    means3D = np.ascontiguousarray(np.asarray(inputs["means3D"], dtype=np.float32))
    opac = np.asarray(inputs["opacities"], dtype=np.float32)
    scales = np.asarray(inputs["scales"], dtype=np.float32)
    cov3D = np.asarray(inputs["cov3D"], dtype=np.float32)
    pc_min = np.asarray(inputs["pc_min"], dtype=np.float32)

    P = pts.shape[0]
    G = means3D.shape[0]
    C = opac.shape[1]
    n_blocks = N_CORES * NBC
    assert P == n_blocks * PBLK, (P, n_blocks * PBLK)

    # integer voxel quantities, identical fp32 arithmetic to the reference
    pts_int = np.floor((pts - pc_min[None, :]) / np.float32(GRID)).astype(np.int32)
    means_int = np.floor((means3D - pc_min[None, :]) / np.float32(GRID)).astype(np.int32)
    radii = np.ceil(scales.max(-1) * np.float32(SCALE_MULT) / np.float32(GRID)).astype(np.int32)
    cov6 = cov3D.reshape(G, 9)[:, [0, 4, 8, 1, 5, 2]].astype(np.float64)
    has_offdiag = np.abs(cov6[:, 3:]).max() > 0.0
    NQ = 10 if has_offdiag else 7

    blocks = _kd_blocks(pts_int, n_blocks)

    binfo = []
    for blk in blocks:
        pi = pts_int[blk]
        lo = pi.min(0)
        hi = pi.max(0)
        gsel = np.where(
            (means_int[:, 0] >= lo[0] - radii) & (means_int[:, 0] <= hi[0] + radii)
            & (means_int[:, 1] >= lo[1] - radii) & (means_int[:, 1] <= hi[1] + radii)
            & (means_int[:, 2] >= lo[2] - radii) & (means_int[:, 2] <= hi[2] + radii)
        )[0]
        binfo.append((blk, lo, hi, gsel))

    # per core, order blocks by gaussian count ascending (light blocks land in
    # input half 0, so the first DMA chunk is the smallest), then take the
    # per-slot max across cores as the shared slot capacity.
    per_core = []
    for ci in range(N_CORES):
        core_b = binfo[ci * NBC:(ci + 1) * NBC]
        core_b.sort(key=lambda t: len(t[3]))
        per_core.append(core_b)
    slot_caps = tuple(
        max(len(per_core[ci][s][3]) for ci in range(N_CORES)) for s in range(NBC)
    )
    if _FORCE128:
        slot_caps = tuple(128 for _ in range(NBC))
    perm = np.concatenate([per_core[ci][s][0] for ci in range(N_CORES)
                           for s in range(NBC)])

    R = 3 * NQ + 3 * MOD
    assert R <= 128, R
    FCOLS = NHB * PBLK
    wstart = {}
    hcols = [0, 0]
    for h in range(2):
        off = FCOLS
        for s in range(h * NHB, (h + 1) * NHB):
            wstart[s] = off
            off += slot_caps[s]
        hcols[h] = off
    TOT = hcols[0] + hcols[1]
    jobs = _jobs_of(slot_caps[:NHB]) + _jobs_of(slot_caps[NHB:])
    njobs = len(jobs)
    # per-slot list of (global job index, slot col offset, chunk cap)
    slot_jobs = {s: [] for s in range(NBC)}
    off_in_slot = {s: 0 for s in range(NBC)}
    for gj, (s_h, cap, first, last) in enumerate(
            [(s, c, f, l) for (s, c, f, l) in _jobs_of(slot_caps[:NHB])]
            + [(s + NHB, c, f, l) for (s, c, f, l) in _jobs_of(slot_caps[NHB:])]):
        slot_jobs[s_h].append((gj, off_in_slot[s_h], cap))
        off_in_slot[s_h] += cap

    bq = np.float64(0.5) * GRID  # voxel center scale

    in_maps = []
    for ci in range(N_CORES):
        fw = np.zeros((R, TOT), np.float16)
        opa_m = np.zeros((128, njobs * C), np.float16)
        for s in range(NBC):
            blk, lo, hi, gsel = per_core[ci][s]
            gl = len(gsel)
            cap_s = slot_caps[s]
            h = s // NHB
            base = (0, hcols[0])[h]
            fsl = base + (s - h * NHB) * PBLK
            wsl = base + wstart[s]

            cen = (lo + hi + 1).astype(np.float64) * bq  # block center, meters
            p64 = pts[blk].astype(np.float64) - cen
            m64 = means3D[gsel].astype(np.float64) - cen

            # ---- point features -------------------------------------
            x, y, z = p64[:, 0], p64[:, 1], p64[:, 2]
            if has_offdiag:
                Q = np.stack([x * x, y * y, z * z, x * y, y * z, x * z,
                              x, y, z, np.ones_like(x)])
            else:
                Q = np.stack([x * x, y * y, z * z, x, y, z, np.ones_like(x)])
            Qh, Qm = _split16(Q)
            F = fw[:, fsl:fsl + PBLK]
            F[0:NQ] = Qh
            F[NQ:2 * NQ] = Qh
            F[2 * NQ:3 * NQ] = Qm
            tcol = np.arange(PBLK)
            for a in range(3):
                r = 3 * NQ + a * MOD + ((pts_int[blk, a] - lo[a]) % MOD)
                F[r, tcol] = 1.0

            # ---- gaussian coefficients ------------------------------
            a_, b_, c_ = cov6[gsel, 0], cov6[gsel, 1], cov6[gsel, 2]
            pxy, pyz, pxz = cov6[gsel, 3], cov6[gsel, 4], cov6[gsel, 5]
            mx, my, mz = m64[:, 0], m64[:, 1], m64[:, 2]
            Amx = a_ * mx + pxy * my + pxz * mz
            Amy = pxy * mx + b_ * my + pyz * mz
            Amz = pxz * mx + pyz * my + c_ * mz
            mAm = mx * Amx + my * Amy + mz * Amz
            const = -0.5 * mAm - 3.0 * MPEN
            if has_offdiag:
                Wq = np.stack([-0.5 * a_, -0.5 * b_, -0.5 * c_,
                               -pxy, -pyz, -pxz, Amx, Amy, Amz, const])
            else:
                Wq = np.stack([-0.5 * a_, -0.5 * b_, -0.5 * c_,
                               Amx, Amy, Amz, const])
            Wh, Wm = _split16(Wq)
            W = fw[:, wsl:wsl + cap_s]
            W[0:NQ, :gl] = Wh
            W[NQ:2 * NQ, :gl] = Wm
            W[2 * NQ:3 * NQ, :gl] = Wh
            W[NQ - 1, gl:] = np.float16(-3.0 * MPEN)  # padded: exp(-672)==0
            gc = np.arange(gl)
            for a in range(3):
                blo = means_int[gsel, a] - radii[gsel]
                bhi = means_int[gsel, a] + radii[gsel]
                for v in range(lo[a], hi[a] + 1):
                    r = 3 * NQ + a * MOD + ((v - lo[a]) % MOD)
                    W[r, gc[(blo <= v) & (v <= bhi)]] = np.float16(MPEN)

            # ---- opacities ------------------------------------------
            for gj, o, cap_j in slot_jobs[s]:
                seg = gsel[o:o + cap_j]
                opa_m[: len(seg), gj * C:(gj + 1) * C] = opac[seg].astype(np.float16)

        in_maps.append({"fw": fw, "opa": opa_m})

    return in_maps, perm, (P, slot_caps, C, R)


def _run(inputs, trace=False, **run_kwargs):
    in_maps, perm, (P, slot_caps, C, R) = _prepare(inputs)
    key = (R, slot_caps, C)
    if key not in _nc_cache:
        _nc_cache[key] = _build_bass(R, slot_caps, C)
    nc = _nc_cache[key]
    try:
        res = run_bass_kernel_spmd(
            nc, in_maps, core_ids=list(range(N_CORES)), trace=trace, **run_kwargs
        )
    except ModuleNotFoundError:
        res = run_bass_kernel_spmd(
            nc, in_maps, core_ids=list(range(N_CORES)), trace=False, **run_kwargs
        )
    out = np.empty((P, C), np.float32)
    for ci in range(N_CORES):
        o = res.results[ci]["out"]  # [128, NBC*C]
        for bi in range(NBC):
            rows = perm[(ci * NBC + bi) * PBLK:(ci * NBC + bi + 1) * PBLK]
            out[rows] = o[:, bi * C:(bi + 1) * C]
    return out, res


def kernel(**inputs):
    return _run(inputs)[0]


# revision 17
# speedup vs baseline: 4.1015x; 1.0439x over previous
"""Trainium2 Bass kernel for the LocalAggregator nn.Module.

Reference computation:
    power[p,g]  = -0.5 * d^T Prec_g d          (d = pts[p] - means3D[g])
    within[p,g] = all(|voxel(pts[p]) - voxel(means3D[g])| <= radii[g])
    logits      = where(within & power<=0, exp(power), 0) @ opacities

Device algorithm:
  * Points are split into 128-point spatial blocks by a recursive KD
    median split; each core owns 16 blocks.  Per block only the
    gaussians whose voxel box overlaps the block bbox are kept
    (<=G_pad of 2048), so the dense pair work per core is
    16 * 128 * G_pad instead of 2048 * 2048.
  * power is a quadratic polynomial in the point coordinates:
    matmul of per-point quadratic features against per-(block,gaussian)
    coefficient columns.  Both sides are stored as two-level fp16
    splits (hi+mid); the three >=2^-22 cross products [Qh*Wh + Qh*Wm +
    Qm*Wh] are computed by stacking rows, so a single 1-cycle/row fp16
    matmul gives ~22-bit precision.
  * the voxel box test is folded into the same matmul with one-hot
    rows over (voxel - lo) mod 16 per axis: contribution 224 per
    within-axis, with -3*224 folded into the constant coefficient, so
    not-within pairs get power <= -224+eps and exp underflows to 0 in
    fp32 (matching the reference's hard mask; Prec is PSD so true
    power <= 0).  The mod-16 aliasing is safe: an aliased pair is
    >=5 m away on that axis, so power <= -25 and exp(power) < 1e-11.
  * ScalarE evaluates exp from PSUM into fp16 weights; a second matmul
    per 128-point block contracts weights against opacities with the
    points as the PSUM partition axis:  logits[p, c] += wt^T . opa.
    All 16 blocks' logits live in a single PSUM bank; one DVE copy and
    one DMA drain the core's whole output.
"""

import numpy as np

import concourse.bass as bass
import concourse.mybir as mybir
import concourse.tile as tile
import concourse.bass2jax as _bass2jax
import concourse.bass_utils as _bass_utils
from concourse.bass_utils import run_bass_kernel_spmd

import json as _json


def _split_waits(bir_json):
    """Walrus in this toolchain rejects instructions carrying more than one
    sync wait ("Too many sync wait commands").  Split every multi-wait
    instruction into a chain of single-wait NoOps on the same engine (program
    order on the engine's sequencer preserves the wait-before-op semantics)."""
    if isinstance(bir_json, (bytes, bytearray)):
        m = _json.loads(bir_json.decode())
    else:
        m = _json.loads(bir_json)
    cnt = 0
    for f in m["functions"]:
        for bb in f["blocks"]:
            new_insts = []
            for inst in bb["instructions"]:
                si = inst.get("sync_info")
                waits = (si or {}).get("on_wait") or []
                if len(waits) > 1:
                    eng = inst.get("engine")
                    for w in waits[:-1]:
                        cnt += 1
                        nop = {
                            "debug": 16,
                            "ins": [],
                            "name": f"I-nopw-{cnt}",
                            "opcode": "NoOp",
                            "outs": [],
                            "sync_info": {"on_update": [], "on_wait": [w]},
                        }
                        if eng is not None:
                            nop["engine"] = eng
                        new_insts.append(nop)
                    si["on_wait"] = [waits[-1]]
                new_insts.append(inst)
            bb["instructions"] = new_insts
    return _json.dumps(m).encode()


_orig_compile_bir_kernel = _bass_utils.compile_bir_kernel.__wrapped__ if hasattr(
    _bass_utils.compile_bir_kernel, "__wrapped__") else _bass_utils.compile_bir_kernel


def _patched_compile_bir_kernel(bir_json, tmpdir, neff_name="file.neff"):
    return _orig_compile_bir_kernel(_split_waits(bir_json), tmpdir, neff_name)


_bass2jax.compile_bir_kernel = _patched_compile_bir_kernel
_bass_utils.compile_bir_kernel = _patched_compile_bir_kernel

GRID = np.float64(0.5)
SCALE_MULT = np.float64(3.0)
MPEN = 224.0  # penalty unit; exact in fp16, and 224 > 104 (fp32 exp underflow)
N_CORES = 8
PBLK = 128  # points per spatial block
NBC = 16  # blocks per core
MOD = 16  # one-hot modulo per axis (aliased pairs are >=5m away -> exp==0)

_nc_cache = {}
_FORCE128 = False


N_WARM = 0  # dummy matmuls keeping the PE p-state ramp warm during DMA wait
NHB = NBC // 2  # block slots per input half


def _jobs_of(slot_caps):
    """Expand per-slot gaussian capacities into (slot, cap, first, last) jobs
    of <=128 gaussians each.  Jobs are ordered cap-descending so the matmul
    that OPENS each PSUM accumulation region (start=True) covers at least the
    partition range of every later matmul in that region; otherwise rows
    beyond the first matmul's partition count would accumulate onto
    uninitialized PSUM."""
    jobs = []
    for s, cap in enumerate(slot_caps):
        chunks = []
        left = cap
        while left > 0:
            take = min(128, left)
            chunks.append(take)
            left -= take
        for i, ch in enumerate(chunks):
            jobs.append((s, ch, i == 0, i == len(chunks) - 1))
    jobs.sort(key=lambda t: -t[1])
    return jobs


def _build_bass(R, slot_caps, C):
    """One core's program.  R = feature rows (<=128), slot_caps = per-block
    gaussian capacities (len NBC; slots 0..7 = input half 0), C = channels."""
    f16 = mybir.dt.float16
    f32 = mybir.dt.float32
    # per-half fw layout: [f (NHB*PBLK) | w slot (slot_caps)]
    FCOLS = NHB * PBLK
    wstart = {}
    hcols = [0, 0]
    for h in range(2):
        off = FCOLS
        for s in range(h * NHB, (h + 1) * NHB):
            wstart[s] = off
            off += slot_caps[s]
        hcols[h] = off
    TOT = hcols[0] + hcols[1]
    half_jobs = [_jobs_of(slot_caps[:NHB]), _jobs_of(slot_caps[NHB:])]
    njobs = len(half_jobs[0]) + len(half_jobs[1])

    nc = bass.Bass()
    fw_d = nc.dram_tensor("fw", [R, TOT], f16, kind="ExternalInput")
    opa_d = nc.dram_tensor("opa", [128, njobs * C], f16, kind="ExternalInput")
    out_d = nc.dram_tensor("out", [128, NBC * C], f32, kind="ExternalOutput")

    ppbufs = 2 if max(len(half_jobs[0]), len(half_jobs[1])) <= 12 else 1
    with tile.TileContext(nc) as tc:
        with (
            tc.tile_pool(name="singles", bufs=1) as singles,
            tc.tile_pool(name="pp", bufs=ppbufs, space="PSUM") as pp,
            tc.tile_pool(name="pl", bufs=1, space="PSUM") as pl,
        ):
            fw_sb = singles.tile([R, TOT], f16)
            opa_sb = singles.tile([128, njobs * C], f16)
            wt = singles.tile([128, njobs * PBLK], f16)
            osb = singles.tile([128, NBC * C], f32)
            psl = pl.tile([128, NBC * C], f32, name="psl")

            nc.sync.dma_start(out=fw_sb[:, :hcols[0]], in_=fw_d[:, :hcols[0]])
            nc.sync.dma_start(out=fw_sb[:, hcols[0]:], in_=fw_d[:, hcols[0]:])
            nc.sync.dma_start(out=opa_sb[:], in_=opa_d[:])

            gj0 = 0
            for h in range(2):
                base = (0, hcols[0])[h]
                jobs = half_jobs[h]
                nh = len(jobs)
                psp = pp.tile([128, nh * PBLK], f32, name="psp")
                woff = {}
                for j, (s_h, cap, first, last) in enumerate(jobs):
                    s = s_h + h * NHB
                    o = woff.get(s, wstart[s])
                    woff[s] = o + cap
                    blk_h = s_h
                    fsl = slice(base + blk_h * PBLK, base + (blk_h + 1) * PBLK)
                    wsl = slice(base + o, base + o + cap)
                    nc.tensor.matmul(
                        psp[:cap, j * PBLK:(j + 1) * PBLK],
                        fw_sb[:, wsl], fw_sb[:, fsl],
                        start=(j % 4 == 0), stop=(j % 4 == 3 or j == nh - 1),
                    )
                nc.scalar.activation(
                    out=wt[:, gj0 * PBLK:(gj0 + nh) * PBLK], in_=psp[:],
                    func=mybir.ActivationFunctionType.Exp,
                )
                for j, (s_h, cap, first, last) in enumerate(jobs):
                    s = s_h + h * NHB
                    gj = gj0 + j
                    nc.tensor.matmul(
                        psl[:, s * C:(s + 1) * C],
                        wt[:cap, gj * PBLK:(gj + 1) * PBLK],
                        opa_sb[:cap, gj * C:(gj + 1) * C],
                        start=(j == 0), stop=(j == nh - 1),
                    )
                gj0 += nh
                hsl = slice(h * NHB * C, (h + 1) * NHB * C)
                nc.vector.tensor_copy(out=osb[:, hsl], in_=psl[:, hsl])
                nc.sync.dma_start(out=out_d[:, hsl], in_=osb[:, hsl])
    return nc


def _kd_blocks(pts_int, n_blocks):
    """Recursive median split on the widest voxel axis -> equal-size blocks."""
    depth = int(np.log2(n_blocks))
    assert (1 << depth) == n_blocks
    blocks = []

    def rec(idx, d):
        if d == 0:
            blocks.append(idx)
            return
        pi = pts_int[idx]
        ax = int(np.argmax(pi.max(0) - pi.min(0)))
        o = idx[np.argsort(pi[:, ax], kind="stable")]
        half = len(o) // 2
        rec(o[:half], d - 1)
        rec(o[half:], d - 1)

    rec(np.arange(len(pts_int)), depth)
    return blocks


def _split16(v):
    """Two-level fp16 split: v ~= hi + mid with ~22-bit mantissa coverage."""
    hi = v.astype(np.float16)
    mid = (v - hi.astype(np.float64)).astype(np.float16)
    return hi, mid


def _prepare(inputs):
    """Host-side prep: KD sharding, per-block gaussian sets, feature and
    coefficient matrices.  All O(P + n_blocks * G)."""
    pts = np.ascontiguousarray(np.asarray(inputs["pts"], dtype=np.float32))
    means3D = np.ascontiguousarray(np.asarray(inputs["means3D"], dtype=np.float32))
    opac = np.asarray(inputs["opacities"], dtype=np.float32)
    scales = np.asarray(inputs["scales"], dtype=np.float32)
    cov3D = np.asarray(inputs["cov3D"], dtype=np.float32)
    pc_min = np.asarray(inputs["pc_min"], dtype=np.float32)

    P = pts.shape[0]
    G = means3D.shape[0]
    C = opac.shape[1]
    n_blocks = N_CORES * NBC
    assert P == n_blocks * PBLK, (P, n_blocks * PBLK)

    # integer voxel quantities, identical fp32 arithmetic to the reference
    pts_int = np.floor((pts - pc_min[None, :]) / np.float32(GRID)).astype(np.int32)
    means_int = np.floor((means3D - pc_min[None, :]) / np.float32(GRID)).astype(np.int32)
    radii = np.ceil(scales.max(-1) * np.float32(SCALE_MULT) / np.float32(GRID)).astype(np.int32)
    cov6 = cov3D.reshape(G, 9)[:, [0, 4, 8, 1, 5, 2]].astype(np.float64)
    has_offdiag = np.abs(cov6[:, 3:]).max() > 0.0
    NQ = 10 if has_offdiag else 7

    blocks = _kd_blocks(pts_int, n_blocks)

    binfo = []
    for blk in blocks:
        pi = pts_int[blk]
        lo = pi.min(0)
        hi = pi.max(0)
        gsel = np.where(
            (means_int[:, 0] >= lo[0] - radii) & (means_int[:, 0] <= hi[0] + radii)
            & (means_int[:, 1] >= lo[1] - radii) & (means_int[:, 1] <= hi[1] + radii)
            & (means_int[:, 2] >= lo[2] - radii) & (means_int[:, 2] <= hi[2] + radii)
        )[0]
        binfo.append((blk, lo, hi, gsel))

    # per core, order blocks by gaussian count ascending (light blocks land in
    # input half 0, so the first DMA chunk is the smallest), then take the
    # per-slot max across cores as the shared slot capacity.
    per_core = []
    for ci in range(N_CORES):
        core_b = binfo[ci * NBC:(ci + 1) * NBC]
        core_b.sort(key=lambda t: len(t[3]))
        per_core.append(core_b)
    slot_caps = tuple(
        max(len(per_core[ci][s][3]) for ci in range(N_CORES)) for s in range(NBC)
    )
    if _FORCE128:
        slot_caps = tuple(128 for _ in range(NBC))
    perm = np.concatenate([per_core[ci][s][0] for ci in range(N_CORES)
                           for s in range(NBC)])

    R = 3 * NQ + 3 * MOD
    assert R <= 128, R
    FCOLS = NHB * PBLK
    wstart = {}
    hcols = [0, 0]
    for h in range(2):
        off = FCOLS
        for s in range(h * NHB, (h + 1) * NHB):
            wstart[s] = off
            off += slot_caps[s]
        hcols[h] = off
    TOT = hcols[0] + hcols[1]
    jobs = _jobs_of(slot_caps[:NHB]) + _jobs_of(slot_caps[NHB:])
    njobs = len(jobs)
    # per-slot list of (global job index, slot col offset, chunk cap)
    slot_jobs = {s: [] for s in range(NBC)}
    off_in_slot = {s: 0 for s in range(NBC)}
    for gj, (s_h, cap, first, last) in enumerate(
            [(s, c, f, l) for (s, c, f, l) in _jobs_of(slot_caps[:NHB])]
            + [(s + NHB, c, f, l) for (s, c, f, l) in _jobs_of(slot_caps[NHB:])]):
        slot_jobs[s_h].append((gj, off_in_slot[s_h], cap))
        off_in_slot[s_h] += cap

    bq = np.float64(0.5) * GRID  # voxel center scale

    in_maps = []
    for ci in range(N_CORES):
        fw = np.zeros((R, TOT), np.float16)
        opa_m = np.zeros((128, njobs * C), np.float16)
        for s in range(NBC):
            blk, lo, hi, gsel = per_core[ci][s]
            gl = len(gsel)
            cap_s = slot_caps[s]
            h = s // NHB
            base = (0, hcols[0])[h]
            fsl = base + (s - h * NHB) * PBLK
            wsl = base + wstart[s]

            cen = (lo + hi + 1).astype(np.float64) * bq  # block center, meters
            p64 = pts[blk].astype(np.float64) - cen
            m64 = means3D[gsel].astype(np.float64) - cen

            # ---- point features -------------------------------------
            x, y, z = p64[:, 0], p64[:, 1], p64[:, 2]
            if has_offdiag:
                Q = np.stack([x * x, y * y, z * z, x * y, y * z, x * z,
                              x, y, z, np.ones_like(x)])
            else:
                Q = np.stack([x * x, y * y, z * z, x, y, z, np.ones_like(x)])
            Qh, Qm = _split16(Q)
            F = fw[:, fsl:fsl + PBLK]
            F[0:NQ] = Qh
            F[NQ:2 * NQ] = Qh
            F[2 * NQ:3 * NQ] = Qm
            tcol = np.arange(PBLK)
            for a in range(3):
                r = 3 * NQ + a * MOD + ((pts_int[blk, a] - lo[a]) % MOD)
                F[r, tcol] = 1.0

            # ---- gaussian coefficients ------------------------------
            a_, b_, c_ = cov6[gsel, 0], cov6[gsel, 1], cov6[gsel, 2]
            pxy, pyz, pxz = cov6[gsel, 3], cov6[gsel, 4], cov6[gsel, 5]
            mx, my, mz = m64[:, 0], m64[:, 1], m64[:, 2]
            Amx = a_ * mx + pxy * my + pxz * mz
            Amy = pxy * mx + b_ * my + pyz * mz
            Amz = pxz * mx + pyz * my + c_ * mz
            mAm = mx * Amx + my * Amy + mz * Amz
            const = -0.5 * mAm - 3.0 * MPEN
            if has_offdiag:
                Wq = np.stack([-0.5 * a_, -0.5 * b_, -0.5 * c_,
                               -pxy, -pyz, -pxz, Amx, Amy, Amz, const])
            else:
                Wq = np.stack([-0.5 * a_, -0.5 * b_, -0.5 * c_,
                               Amx, Amy, Amz, const])
            Wh, Wm = _split16(Wq)
            W = fw[:, wsl:wsl + cap_s]
            W[0:NQ, :gl] = Wh
            W[NQ:2 * NQ, :gl] = Wm
            W[2 * NQ:3 * NQ, :gl] = Wh
            W[NQ - 1, gl:] = np.float16(-3.0 * MPEN)  # padded: exp(-672)==0
            gc = np.arange(gl)
            for a in range(3):
                blo = means_int[gsel, a] - radii[gsel]
                bhi = means_int[gsel, a] + radii[gsel]
                for v in range(lo[a], hi[a] + 1):
                    r = 3 * NQ + a * MOD + ((v - lo[a]) % MOD)
                    W[r, gc[(blo <= v) & (v <= bhi)]] = np.float16(MPEN)

            # ---- opacities ------------------------------------------
            for gj, o, cap_j in slot_jobs[s]:
                seg = gsel[o:o + cap_j]
                opa_m[: len(seg), gj * C:(gj + 1) * C] = opac[seg].astype(np.float16)

        in_maps.append({"fw": fw, "opa": opa_m})

    return in_maps, perm, (P, slot_caps, C, R)


def _run(inputs, trace=False, **run_kwargs):
    in_maps, perm, (P, slot_caps, C, R) = _prepare(inputs)
    key = (R, slot_caps, C)
    if key not in _nc_cache:
        _nc_cache[key] = _build_bass(R, slot_caps, C)
    nc = _nc_cache[key]
    try:
        res = run_bass_kernel_spmd(
            nc, in_maps, core_ids=list(range(N_CORES)), trace=trace, **run_kwargs
        )
    except ModuleNotFoundError:
        res = run_bass_kernel_spmd(
            nc, in_maps, core_ids=list(range(N_CORES)), trace=False, **run_kwargs
        )
    out = np.empty((P, C), np.float32)
    for ci in range(N_CORES):
        o = res.results[ci]["out"]  # [128, NBC*C]
        for bi in range(NBC):
            rows = perm[(ci * NBC + bi) * PBLK:(ci * NBC + bi + 1) * PBLK]
            out[rows] = o[:, bi * C:(bi + 1) * C]
    return out, res


def kernel(**inputs):
    return _run(inputs)[0]


# revision 18
# speedup vs baseline: 4.2560x; 1.0377x over previous
"""Trainium2 Bass kernel for the LocalAggregator nn.Module.

Reference computation:
    power[p,g]  = -0.5 * d^T Prec_g d          (d = pts[p] - means3D[g])
    within[p,g] = all(|voxel(pts[p]) - voxel(means3D[g])| <= radii[g])
    logits      = where(within & power<=0, exp(power), 0) @ opacities

Device algorithm:
  * Points are split into 128-point spatial blocks by a recursive KD
    median split; each core owns 16 blocks.  Per block only the
    gaussians whose voxel box overlaps the block bbox are kept, so the
    dense pair work per core is ~16*128*cap instead of 2048*2048.
  * power is a quadratic polynomial in the point coordinates:
    matmul of per-point quadratic features against per-(block,gaussian)
    coefficient columns.  Both sides are stored as two-level fp16
    splits (hi+mid); the three >=2^-22 cross products [Qh*Wh + Qh*Wm +
    Qm*Wh] are computed by stacking rows, so a single 1-cycle/row fp16
    matmul gives ~22-bit precision.
  * the voxel box test is folded into the same matmul with one-hot
    rows over (voxel - lo) mod M per axis: contribution 224 per
    within-axis, with -3*224 folded into the constant coefficient, so
    not-within pairs get power <= -224+eps and exp underflows to 0 in
    fp32 (matching the reference's hard mask; Prec is PSD so true
    power <= 0).  M per axis is the exact block span when small, else
    an alias-safe modulus: an aliased pair is >= (M-r-1)/2 meters away
    on that axis, so exp(power) < 1e-4 -> negligible vs the 2e-2 gate.
  * ScalarE evaluates exp from PSUM into fp16 weights; a second matmul
    per block contracts weights against opacities with the points as
    the PSUM partition axis: logits[p, c] += wt^T . opa.  Two blocks
    with <=64 gaussians each share one 128-col pair window at PSUM
    partition offsets 0/64, shrinking the exp width.  All 16 blocks'
    logits live in a single PSUM bank; per input half one DVE copy and
    one DMA drain the output, overlapping the second half's compute.
"""

import numpy as np

import concourse.bass as bass
import concourse.mybir as mybir
import concourse.tile as tile
import concourse.bass2jax as _bass2jax
import concourse.bass_utils as _bass_utils
from concourse.bass_utils import run_bass_kernel_spmd

import json as _json


def _split_waits(bir_json):
    """Walrus in this toolchain rejects instructions carrying more than one
    sync wait ("Too many sync wait commands").  Split every multi-wait
    instruction into a chain of single-wait NoOps on the same engine (program
    order on the engine's sequencer preserves the wait-before-op semantics)."""
    if isinstance(bir_json, (bytes, bytearray)):
        m = _json.loads(bir_json.decode())
    else:
        m = _json.loads(bir_json)
    cnt = 0
    for f in m["functions"]:
        for bb in f["blocks"]:
            new_insts = []
            for inst in bb["instructions"]:
                si = inst.get("sync_info")
                waits = (si or {}).get("on_wait") or []
                if len(waits) > 1:
                    eng = inst.get("engine")
                    for w in waits[:-1]:
                        cnt += 1
                        nop = {
                            "debug": 16,
                            "ins": [],
                            "name": f"I-nopw-{cnt}",
                            "opcode": "NoOp",
                            "outs": [],
                            "sync_info": {"on_update": [], "on_wait": [w]},
                        }
                        if eng is not None:
                            nop["engine"] = eng
                        new_insts.append(nop)
                    si["on_wait"] = [waits[-1]]
                new_insts.append(inst)
            bb["instructions"] = new_insts
    return _json.dumps(m).encode()


_orig_compile_bir_kernel = _bass_utils.compile_bir_kernel.__wrapped__ if hasattr(
    _bass_utils.compile_bir_kernel, "__wrapped__") else _bass_utils.compile_bir_kernel


def _patched_compile_bir_kernel(bir_json, tmpdir, neff_name="file.neff"):
    return _orig_compile_bir_kernel(_split_waits(bir_json), tmpdir, neff_name)


_bass2jax.compile_bir_kernel = _patched_compile_bir_kernel
_bass_utils.compile_bir_kernel = _patched_compile_bir_kernel

GRID = np.float64(0.5)
SCALE_MULT = np.float64(3.0)
MPEN = 224.0  # penalty unit; exact in fp16, and 224 > 104 (fp32 exp underflow)
N_CORES = 8
PBLK = 128  # points per spatial block
NBC = 16  # blocks per core
NHB = NBC // 2  # block slots per input half

_nc_cache = {}


def _layout(slot_caps):
    """Shared host/program layout.

    Returns (wstart, hcols, TOT, halves, njobs) where halves[h] is a list of
    windows and each window is a list of jobs
    (slot, chunk_off, cap, poff, gj, first, last):
      - slot: global block slot (0..NBC-1)
      - chunk_off: gaussian offset inside the slot (chunks of <=128)
      - cap: gaussians this job covers
      - poff: PSUM/wt/opa partition offset (0, or 64 for the second job of a
        packed window; packing requires both caps <= 64)
      - gj: flat job index (opa column group)
      - first/last: chunk position within the slot (psl start/stop flags)
    """
    FCOLS = NHB * PBLK
    wstart = {}
    hcols = [0, 0]
    for h in range(2):
        off = FCOLS
        for s in range(h * NHB, (h + 1) * NHB):
            wstart[s] = off
            off += slot_caps[s]
        hcols[h] = off
    TOT = hcols[0] + hcols[1]

    halves = []
    gj = 0
    for h in range(2):
        jobs = []
        for s in range(h * NHB, (h + 1) * NHB):
            cap = slot_caps[s]
            off = 0
            while cap > 0:
                take = min(128, cap)
                jobs.append([s, off, take, off == 0, cap - take == 0])
                off += take
                cap -= take
        small = [j for j in jobs if j[2] <= 64]
        big = [j for j in jobs if j[2] > 64]
        wins = []
        while len(small) >= 2:
            a = small.pop(0)
            b = small.pop()
            wins.append([(a, 0), (b, 64)])
        for j in small + big:
            wins.append([(j, 0)])
        out_wins = []
        for win in wins:
            jw = []
            for (s, coff, cap, first, last), poff in [
                (t[0], t[1]) if isinstance(t, tuple) else (t, 0) for t in win
            ]:
                jw.append((s, coff, cap, poff, gj, first, last))
                gj += 1
            out_wins.append(jw)
        halves.append(out_wins)
    return wstart, hcols, TOT, halves, gj


def _build_bass(R, slot_caps, C):
    """One core's program.  R = feature rows (<=128), slot_caps = per-block
    gaussian capacities (len NBC; slots 0..7 = input half 0), C = channels."""
    f16 = mybir.dt.float16
    f32 = mybir.dt.float32
    wstart, hcols, TOT, halves, njobs = _layout(slot_caps)
    nwin = [len(halves[0]), len(halves[1])]

    nc = bass.Bass()
    fw_d = nc.dram_tensor("fw", [R, TOT], f16, kind="ExternalInput")
    opa_d = nc.dram_tensor("opa", [128, njobs * C], f16, kind="ExternalInput")
    out_d = nc.dram_tensor("out", [128, NBC * C], f32, kind="ExternalOutput")

    ppbufs = 2 if max(nwin) <= 12 else 1
    with tile.TileContext(nc) as tc:
        with (
            tc.tile_pool(name="singles", bufs=1) as singles,
            tc.tile_pool(name="pp", bufs=ppbufs, space="PSUM") as pp,
            tc.tile_pool(name="pl", bufs=1, space="PSUM") as pl,
        ):
            fw_sb = singles.tile([R, TOT], f16)
            opa_sb = singles.tile([128, njobs * C], f16)
            wt = singles.tile([128, (nwin[0] + nwin[1]) * PBLK], f16)
            osb = singles.tile([128, NBC * C], f32)
            psl = pl.tile([128, NBC * C], f32, name="psl")

            nc.sync.dma_start(out=fw_sb[:, :hcols[0]], in_=fw_d[:, :hcols[0]])
            nc.sync.dma_start(out=fw_sb[:, hcols[0]:], in_=fw_d[:, hcols[0]:])
            nc.sync.dma_start(out=opa_sb[:], in_=opa_d[:])

            wi0 = 0
            for h in range(2):
                base = (0, hcols[0])[h]
                wins = halves[h]
                psp = pp.tile([128, nwin[h] * PBLK], f32, name="psp")
                for wi, win in enumerate(wins):
                    for (s, coff, cap, poff, gj, first, last) in win:
                        blk_h = s - h * NHB
                        fsl = slice(base + blk_h * PBLK, base + (blk_h + 1) * PBLK)
                        wo = base + wstart[s] + coff
                        nc.tensor.matmul(
                            psp[poff:poff + cap, wi * PBLK:(wi + 1) * PBLK],
                            fw_sb[:, wo:wo + cap], fw_sb[:, fsl],
                            start=True, stop=True,
                        )
                nc.scalar.activation(
                    out=wt[:, wi0 * PBLK:(wi0 + nwin[h]) * PBLK], in_=psp[:],
                    func=mybir.ActivationFunctionType.Exp,
                )
                for wi, win in enumerate(wins):
                    for (s, coff, cap, poff, gj, first, last) in win:
                        wtc = (wi0 + wi) * PBLK
                        nc.tensor.matmul(
                            psl[:, s * C:(s + 1) * C],
                            wt[poff:poff + cap, wtc:wtc + PBLK],
                            opa_sb[poff:poff + cap, gj * C:(gj + 1) * C],
                            start=first, stop=last,
                        )
                wi0 += nwin[h]
                hsl = slice(h * NHB * C, (h + 1) * NHB * C)
                nc.vector.tensor_copy(out=osb[:, hsl], in_=psl[:, hsl])
                nc.sync.dma_start(out=out_d[:, hsl], in_=osb[:, hsl])
    return nc


def _kd_blocks(pts_int, n_blocks):
    """Recursive median split on the widest voxel axis -> equal-size blocks."""
    depth = int(np.log2(n_blocks))
    assert (1 << depth) == n_blocks
    blocks = []

    def rec(idx, d):
        if d == 0:
            blocks.append(idx)
            return
        pi = pts_int[idx]
        ax = int(np.argmax(pi.max(0) - pi.min(0)))
        o = idx[np.argsort(pi[:, ax], kind="stable")]
        half = len(o) // 2
        rec(o[:half], d - 1)
        rec(o[half:], d - 1)

    rec(np.arange(len(pts_int)), depth)
    return blocks


def _split16(v):
    """Two-level fp16 split: v ~= hi + mid with ~22-bit mantissa coverage."""
    hi = v.astype(np.float16)
    mid = (v - hi.astype(np.float64)).astype(np.float16)
    return hi, mid


def _prepare(inputs):
    """Host-side prep: KD sharding, per-block gaussian sets, feature and
    coefficient matrices.  All O(P + n_blocks * G)."""
    pts = np.ascontiguousarray(np.asarray(inputs["pts"], dtype=np.float32))
    means3D = np.ascontiguousarray(np.asarray(inputs["means3D"], dtype=np.float32))
    opac = np.asarray(inputs["opacities"], dtype=np.float32)
    scales = np.asarray(inputs["scales"], dtype=np.float32)
    cov3D = np.asarray(inputs["cov3D"], dtype=np.float32)
    pc_min = np.asarray(inputs["pc_min"], dtype=np.float32)

    P = pts.shape[0]
    G = means3D.shape[0]
    C = opac.shape[1]
    n_blocks = N_CORES * NBC
    assert P == n_blocks * PBLK, (P, n_blocks * PBLK)

    # integer voxel quantities, identical fp32 arithmetic to the reference
    pts_int = np.floor((pts - pc_min[None, :]) / np.float32(GRID)).astype(np.int32)
    means_int = np.floor((means3D - pc_min[None, :]) / np.float32(GRID)).astype(np.int32)
    radii = np.ceil(scales.max(-1) * np.float32(SCALE_MULT) / np.float32(GRID)).astype(np.int32)
    cov6 = cov3D.reshape(G, 9)[:, [0, 4, 8, 1, 5, 2]].astype(np.float64)
    has_offdiag = np.abs(cov6[:, 3:]).max() > 0.0
    NQ = 10 if has_offdiag else 7

    blocks = _kd_blocks(pts_int, n_blocks)

    binfo = []
    for blk in blocks:
        pi = pts_int[blk]
        lo = pi.min(0)
        hi = pi.max(0)
        gsel = np.where(
            (means_int[:, 0] >= lo[0] - radii) & (means_int[:, 0] <= hi[0] + radii)
            & (means_int[:, 1] >= lo[1] - radii) & (means_int[:, 1] <= hi[1] + radii)
            & (means_int[:, 2] >= lo[2] - radii) & (means_int[:, 2] <= hi[2] + radii)
        )[0]
        binfo.append((blk, lo, hi, gsel))

    # one-hot modulus per axis: exact span when small, else alias-safe (an
    # aliased pair is >= (M-r-1)*GRID/... meters away -> exp underflows)
    rmax = int(radii.max())
    m_alias = max(2 * rmax + 2, rmax + 9)
    span_max = np.array([max(t[2][a] - t[1][a] + 1 for t in binfo) for a in range(3)])
    Ms = [int(span_max[a]) if span_max[a] <= max(16, m_alias) else m_alias
          for a in range(3)]
    Moff = [3 * NQ, 3 * NQ + Ms[0], 3 * NQ + Ms[0] + Ms[1]]
    R = 3 * NQ + sum(Ms)
    assert R <= 128, R

    # per core, order blocks by gaussian count ascending (light blocks land in
    # input half 0, so the first DMA chunk is the smallest), then take the
    # per-slot max across cores as the shared slot capacity.
    per_core = []
    for ci in range(N_CORES):
        core_b = binfo[ci * NBC:(ci + 1) * NBC]
        core_b.sort(key=lambda t: len(t[3]))
        per_core.append(core_b)
    slot_caps = tuple(
        max(len(per_core[ci][s][3]) for ci in range(N_CORES)) for s in range(NBC)
    )
    perm = np.concatenate([per_core[ci][s][0] for ci in range(N_CORES)
                           for s in range(NBC)])

    wstart, hcols, TOT, halves, njobs = _layout(slot_caps)
    # per-slot job list: (gj, chunk_off, cap, poff)
    slot_jobs = {s: [] for s in range(NBC)}
    for h in range(2):
        for win in halves[h]:
            for (s, coff, cap, poff, gj, first, last) in win:
                slot_jobs[s].append((gj, coff, cap, poff))

    bq = np.float64(0.5) * GRID  # voxel center scale

    in_maps = []
    for ci in range(N_CORES):
        fw = np.zeros((R, TOT), np.float16)
        opa_m = np.zeros((128, njobs * C), np.float16)
        for s in range(NBC):
            blk, lo, hi, gsel = per_core[ci][s]
            gl = len(gsel)
            cap_s = slot_caps[s]
            h = s // NHB
            base = (0, hcols[0])[h]
            fsl = base + (s - h * NHB) * PBLK
            wsl = base + wstart[s]

            cen = (lo + hi + 1).astype(np.float64) * bq  # block center, meters
            p64 = pts[blk].astype(np.float64) - cen
            m64 = means3D[gsel].astype(np.float64) - cen

            # ---- point features -------------------------------------
            x, y, z = p64[:, 0], p64[:, 1], p64[:, 2]
            if has_offdiag:
                Q = np.stack([x * x, y * y, z * z, x * y, y * z, x * z,
                              x, y, z, np.ones_like(x)])
            else:
                Q = np.stack([x * x, y * y, z * z, x, y, z, np.ones_like(x)])
            Qh, Qm = _split16(Q)
            F = fw[:, fsl:fsl + PBLK]
            F[0:NQ] = Qh
            F[NQ:2 * NQ] = Qh
            F[2 * NQ:3 * NQ] = Qm
            tcol = np.arange(PBLK)
            for a in range(3):
                r = Moff[a] + ((pts_int[blk, a] - lo[a]) % Ms[a])
                F[r, tcol] = 1.0

            # ---- gaussian coefficients ------------------------------
            a_, b_, c_ = cov6[gsel, 0], cov6[gsel, 1], cov6[gsel, 2]
            pxy, pyz, pxz = cov6[gsel, 3], cov6[gsel, 4], cov6[gsel, 5]
            mx, my, mz = m64[:, 0], m64[:, 1], m64[:, 2]
            Amx = a_ * mx + pxy * my + pxz * mz
            Amy = pxy * mx + b_ * my + pyz * mz
            Amz = pxz * mx + pyz * my + c_ * mz
            mAm = mx * Amx + my * Amy + mz * Amz
            const = -0.5 * mAm - 3.0 * MPEN
            if has_offdiag:
                Wq = np.stack([-0.5 * a_, -0.5 * b_, -0.5 * c_,
                               -pxy, -pyz, -pxz, Amx, Amy, Amz, const])
            else:
                Wq = np.stack([-0.5 * a_, -0.5 * b_, -0.5 * c_,
                               Amx, Amy, Amz, const])
            Wh, Wm = _split16(Wq)
            W = fw[:, wsl:wsl + cap_s]
            W[0:NQ, :gl] = Wh
            W[NQ:2 * NQ, :gl] = Wm
            W[2 * NQ:3 * NQ, :gl] = Wh
            W[NQ - 1, gl:] = np.float16(-3.0 * MPEN)  # padded: exp(-672)==0
            gc = np.arange(gl)
            for a in range(3):
                blo = means_int[gsel, a] - radii[gsel]
                bhi = means_int[gsel, a] + radii[gsel]
                for v in range(lo[a], hi[a] + 1):
                    r = Moff[a] + ((v - lo[a]) % Ms[a])
                    W[r, gc[(blo <= v) & (v <= bhi)]] = np.float16(MPEN)

            # ---- opacities (at the job's partition offset) ----------
            for gj, coff, cap_j, poff in slot_jobs[s]:
                seg = gsel[coff:coff + cap_j]
                opa_m[poff:poff + len(seg), gj * C:(gj + 1) * C] = \
                    opac[seg].astype(np.float16)

        in_maps.append({"fw": fw, "opa": opa_m})

    return in_maps, perm, (P, slot_caps, C, R)


def _run(inputs, trace=False, **run_kwargs):
    in_maps, perm, (P, slot_caps, C, R) = _prepare(inputs)
    key = (R, slot_caps, C)
    if key not in _nc_cache:
        _nc_cache[key] = _build_bass(R, slot_caps, C)
    nc = _nc_cache[key]
    try:
        res = run_bass_kernel_spmd(
            nc, in_maps, core_ids=list(range(N_CORES)), trace=trace, **run_kwargs
        )
    except ModuleNotFoundError:
        res = run_bass_kernel_spmd(
            nc, in_maps, core_ids=list(range(N_CORES)), trace=False, **run_kwargs
        )
    out = np.empty((P, C), np.float32)
    for ci in range(N_CORES):
        o = res.results[ci]["out"]  # [128, NBC*C]
        for bi in range(NBC):
            rows = perm[(ci * NBC + bi) * PBLK:(ci * NBC + bi + 1) * PBLK]
            out[rows] = o[:, bi * C:(bi + 1) * C]
    return out, res


def kernel(**inputs):
    return _run(inputs)[0]
